# revision 1
# baseline (speedup 1.0000x reference)
"""DRNN-Char (4-layer dilated QRNN + decoder) Trainium2 kernel, v3.

Sharding: data-parallel over batch. 16 rows / 8 cores = 2 rows per core.

- Layer 0 is a host-side table lookup (gates depend only on the input token):
  the host sends gathered z'0=(1-f0)*tanh(z0), f0, sigmoid(o0) tables; no L0
  matmul or activations on device.
- Direct C-space recurrence c = f*c + (1-f)*tanh(z). Scans are contiguous
  (each layer's tensors live in that layer's dilation order; the inter-layer
  reorder rides on strided matmul rhs APs) and run in fp32 (measured: the
  serial scan is 1 elem/cycle in fp32, 2x slower in bf16).
- f,o gate matmuls in fp8e4 + DoubleRow (2x PE). z stays bf16 (z errors are
  first-order in the output, f/o only second-order). Scales SW/SX keep fp8
  operands in e4m3 range; the activation scale divides them out.
- Matmuls cover a full [128,1024] PSUM tile in ONE instruction (1024 strided
  columns), so LDWEIGHTS (123ns) hides behind 213-427ns matmuls.
- combine2 (fp8 x-copy) on GpSimd; everything else elementwise on DVE.
"""

import numpy as np
import ml_dtypes

EMB = 256
HID = 512
LAYERS = 4
VOCAB = 256
B = 16
T = 2048
NCORES = 8
BC = B // NCORES
HCH = HID // 128

SW = 32.0                      # fp8 weight scale
SX = [32.0, 128.0, 256.0]      # fp8 x scale for h0,h1,h2 (inputs of L1..L3)

_cache = {}


def _build():
    if "nc" in _cache:
        return _cache["nc"]

    import concourse.bass as bass
    import concourse.mybir as mybir
    import concourse.tile as tile
    from concourse import bacc

    f32 = mybir.dt.float32
    bf16 = mybir.dt.bfloat16
    fp8 = mybir.dt.float8e4
    SIG = mybir.ActivationFunctionType.Sigmoid
    TANH = mybir.ActivationFunctionType.Tanh
    MULT = mybir.AluOpType.mult
    ADD = mybir.AluOpType.add
    SUB = mybir.AluOpType.subtract
    DR = mybir.MatmulPerfMode.DoubleRow

    nc = bacc.Bacc(
        "TRN2",
        target_bir_lowering=False,
        debug=False,
        enable_asserts=False,
        num_devices=NCORES,
    )

    # ---- DRAM inputs (host-prepped, per core) ----
    zp0_d = nc.dram_tensor("zp0", [BC, 4, 128, 4, 512], bf16, kind="ExternalInput").ap()
    f0_d = nc.dram_tensor("f0", [BC, 4, 128, 4, 512], bf16, kind="ExternalInput").ap()
    so0_d = nc.dram_tensor("so0", [BC, 4, 128, 4, 512], bf16, kind="ExternalInput").ap()
    wz_d = nc.dram_tensor("wz", [3, 128, 4, 512], bf16, kind="ExternalInput").ap()
    wfo_d = nc.dram_tensor("wfo", [3, 128, 4, 1024], fp8, kind="ExternalInput").ap()
    wd_d = nc.dram_tensor("wd", [128, 4, VOCAB], bf16, kind="ExternalInput").ap()
    bias_d = nc.dram_tensor("bias", [128, 3, 12], f32, kind="ExternalInput").ap()
    decb_d = nc.dram_tensor("decb", [1, VOCAB], bf16, kind="ExternalInput").ap()
    out_d = nc.dram_tensor("out", [BC, T, VOCAB], f32, kind="ExternalOutput").ap()

    with tile.TileContext(nc) as tc:
        with (
            tc.tile_pool(name="consts", bufs=1) as consts,
            tc.tile_pool(name="acts", bufs=1) as acts,
            tc.tile_pool(name="l0t", bufs=2) as l0t,
            tc.tile_pool(name="stage", bufs=2) as stage,
            tc.tile_pool(name="ccl0", bufs=2) as ccl0,
            tc.tile_pool(name="ccp", bufs=2) as ccp,
            tc.tile_pool(name="zpp", bufs=1) as zpp,
            tc.tile_pool(name="outs", bufs=2) as outs,
            tc.tile_pool(name="psum", bufs=4, space="PSUM") as psum,
        ):
            # ---- resident tiles ----
            wz_sb = [consts.tile([128, 4, 512], bf16, tag=f"wz{i}", name=f"wz{i}") for i in range(3)]
            wfo_sb = [consts.tile([128, 4, 1024], fp8, tag=f"wfo{i}", name=f"wfo{i}") for i in range(3)]
            wd = consts.tile([128, 4, VOCAB], bf16, tag="wd", name="wd")
            bias = consts.tile([128, 3, 12], f32, tag="bias", name="bias")
            decb = consts.tile([1, VOCAB], bf16, tag="decb", name="decb")
            ones = consts.tile([1, 128], bf16, tag="ones", name="ones")

            xbuf = [acts.tile([128, 4, T], bf16, tag=f"x{r}", name=f"x{r}") for r in range(BC)]
            hbuf = [acts.tile([128, 4, T], bf16, tag=f"h{r}", name=f"h{r}") for r in range(BC)]
            x8 = [acts.tile([128, 4, T], fp8, tag=f"x8{r}", name=f"x8{r}") for r in range(BC)]

            # ---- const DMAs ----
            for i in range(3):
                nc.gpsimd.dma_start(wz_sb[i][:], wz_d[i])
                nc.gpsimd.dma_start(wfo_sb[i][:], wfo_d[i])
            nc.gpsimd.dma_start(wd[:], wd_d[:])
            nc.gpsimd.dma_start(bias[:], bias_d[:])
            nc.gpsimd.dma_start(decb[:], decb_d[:])
            nc.gpsimd.memset(ones[:], 1.0)

            # ---- layer 0: scan over host-gathered tables ----
            for r in range(BC):
                prev = [None] * HCH
                for q4 in range(4):
                    zp = l0t.tile([128, 4, 512], bf16, tag="zp", name="zp")
                    f0t = l0t.tile([128, 4, 512], bf16, tag="f0", name="f0")
                    so0t = l0t.tile([128, 4, 512], bf16, tag="so0", name="so0")
                    nc.sync.dma_start(zp[:], zp0_d[r, q4])
                    nc.sync.dma_start(f0t[:], f0_d[r, q4])
                    nc.sync.dma_start(so0t[:], so0_d[r, q4])
                    sl = slice(q4 * 512, q4 * 512 + 512)
                    for h in range(HCH):
                        cq = ccl0.tile([128, 512], f32, tag=f"cc0_{h}", name=f"cc0_{h}")
                        init = 0.0 if q4 == 0 else prev[h][:, 511:512]
                        nc.vector.tensor_tensor_scan(
                            cq[:], f0t[:, h, :], zp[:, h, :],
                            initial=init, op0=MULT, op1=ADD,
                        )
                        prev[h] = cq
                        nc.vector.tensor_tensor(
                            xbuf[r][:, h, sl], so0t[:, h, :], cq[:], MULT
                        )
                        nc.gpsimd.tensor_scalar_mul(
                            x8[r][:, h, sl], xbuf[r][:, h, sl], SX[0]
                        )

            # ---- layers 1..3 ----
            for li in (1, 2, 3):
                idx = li - 1
                rho = 2 ** li
                ascale = 1.0 / (SW * SX[idx])
                for r in range(BC):
                    xin, hout, x8in = xbuf[r], hbuf[r], x8[r]
                    for h in range(HCH):
                        gts = {}
                        # z gate: bf16; ztneg = tanh(-(pre + bz))
                        zt = stage.tile([128, T], bf16, tag="zt", name="zt")
                        for pb in range(2):
                            ps = psum.tile([128, 1024], f32, tag="ps", name="ps")
                            for k in range(4):
                                for u in range(2):
                                    nc.tensor.matmul(
                                        ps[:, u * 512 : (u + 1) * 512],
                                        lhsT=wz_sb[idx][:, k, h * 128 : (h + 1) * 128],
                                        rhs=xin[:, k, pb + u * 1024 : pb + u * 1024 + 1023 : 2],
                                        start=(k == 0),
                                        stop=(k == 3),
                                    )
                            nc.scalar.activation(
                                zt[:, pb * 1024 : (pb + 1) * 1024], ps[:], TANH,
                                bias=bias[:, idx, h : h + 1], scale=-1.0,
                            )
                        gts["z"] = zt
                        # f,o gates: fp8 DoubleRow
                        for g, gname in ((0, "f"), (1, "o")):
                            gdt = f32 if gname == "f" else bf16
                            gt = stage.tile([128, T], gdt, tag=gname, name=gname)
                            for pb in range(2):
                                ps = psum.tile([128, 1024], f32, tag="ps", name="ps")
                                for kp in range(2):
                                    for u in range(2):
                                        nc.tensor.matmul(
                                            ps[:, u * 512 : (u + 1) * 512],
                                            lhsT=wfo_sb[idx][:, 2 * kp : 2 * kp + 2, g * 512 + h * 128 : g * 512 + (h + 1) * 128],
                                            rhs=x8in[:, 2 * kp : 2 * kp + 2, pb + u * 1024 : pb + u * 1024 + 1023 : 2],
                                            start=(kp == 0),
                                            stop=(kp == 1),
                                            perf_mode=DR,
                                        )
                                nc.scalar.activation(
                                    gt[:, pb * 1024 : (pb + 1) * 1024], ps[:], SIG,
                                    bias=bias[:, idx, (g + 1) * 4 + h : (g + 1) * 4 + h + 1],
                                    scale=ascale,
                                )
                            gts[gname] = gt
                        # z' = (f - 1) * (-tanh) = (1-f) tanh -> fp32 for the scan
                        zpt = zpp.tile([128, T], f32, tag="zp", name="zp")
                        nc.vector.scalar_tensor_tensor(
                            zpt[:], gts["f"][:], 1.0, gts["z"][:], SUB, MULT
                        )
                        cc = ccp.tile([128, T], f32, tag="cc", name="cc")
                        for j in range(rho):
                            ssl = slice(j * (T // rho), (j + 1) * (T // rho))
                            nc.vector.tensor_tensor_scan(
                                cc[:, ssl], gts["f"][:, ssl], zpt[:, ssl],
                                initial=0.0, op0=MULT, op1=ADD,
                            )
                        nc.vector.tensor_tensor(hout[:, h, :], gts["o"][:], cc[:], MULT)
                        if li < 3:
                            nc.gpsimd.tensor_scalar_mul(
                                x8in[:, h, :], hout[:, h, :], SX[li]
                            )
                    xbuf[r], hbuf[r] = hbuf[r], xbuf[r]

            # ---- decoder (h3 in dilation-8 order; scatter rows on DMA out) ----
            for r in range(BC):
                xin = xbuf[r]
                for mt in range(T // 128):
                    ps = psum.tile([128, 1024], f32, tag="ps", name="ps")
                    for k in range(4):
                        nc.tensor.matmul(
                            ps[:, 0:VOCAB],
                            lhsT=xin[:, k, mt * 128 : (mt + 1) * 128],
                            rhs=wd[:, k, :],
                            start=(k == 0),
                            stop=False,
                        )
                    nc.tensor.matmul(
                        ps[:, 0:VOCAB], lhsT=ones[:], rhs=decb[:],
                        start=False, stop=True,
                    )
                    ot = outs.tile([128, VOCAB], f32, tag="ot", name="ot")
                    nc.vector.tensor_copy(ot[:], ps[:, 0:VOCAB])
                    # dilation-8 index i = j*256 + q -> t = 8q + j
                    t0 = 1024 * (mt % 2) + mt // 2
                    nc.sync.dma_start(out_d[r, t0 : t0 + 1017 : 8, :], ot[:])

    nc.compile()
    _cache["nc"] = nc
    return nc


def _prep_inputs(inputs):
    bf = ml_dtypes.bfloat16
    f8 = ml_dtypes.float8_e4m3fn
    x = np.asarray(inputs["x"]).astype(np.int64)
    emb = np.asarray(inputs["emb"], dtype=np.float32)
    Ws = [np.asarray(inputs[f"W{i}"], dtype=np.float32) for i in range(LAYERS)]
    bs = [np.asarray(inputs[f"b{i}"], dtype=np.float32) for i in range(LAYERS)]
    decW = np.asarray(inputs["decW"], dtype=np.float32)
    decb = np.asarray(inputs["decb"], dtype=np.float32)

    # layer-0 per-vocab gate tables
    pre0 = emb @ Ws[0] + bs[0]          # [VOCAB, 3H]
    zt0 = np.tanh(pre0[:, :HID])
    f0 = 1.0 / (1.0 + np.exp(-pre0[:, HID : 2 * HID]))
    so0 = 1.0 / (1.0 + np.exp(-pre0[:, 2 * HID :]))
    f0 = f0.astype(bf).astype(np.float32)
    zp0 = ((1.0 - f0) * zt0).astype(bf)
    so0b = so0.astype(bf)

    def table_arrange(tab, idx):
        # tab [VOCAB, HID] -> gathered [T, HID] -> [4(q), 128, 4(k), 512]
        g = tab[idx]                                  # [T, 512]
        return np.ascontiguousarray(
            g.T.reshape(4, 128, 4, 512).transpose(2, 1, 0, 3)
        )

    wz = np.stack(
        [np.ascontiguousarray(Ws[i][:, :HID].reshape(4, 128, 512).transpose(1, 0, 2)).astype(bf) for i in range(1, 4)]
    )
    wfo = np.stack(
        [
            np.ascontiguousarray((Ws[i][:, HID:] * SW).reshape(4, 128, 1024).transpose(1, 0, 2)).astype(f8)
            for i in range(1, 4)
        ]
    )
    wdt = np.ascontiguousarray(decW.reshape(4, 128, VOCAB).transpose(1, 0, 2)).astype(bf)

    bias = np.zeros((128, 3, 12), np.float32)
    for i in range(1, 4):
        bb = bs[i].reshape(3, 4, 128)  # [gate, h, p]
        bias[:, i - 1, 0:4] = -bb[0].T
        bias[:, i - 1, 4:8] = bb[1].T
        bias[:, i - 1, 8:12] = bb[2].T

    decbb = decb.reshape(1, VOCAB).astype(bf)

    in_maps = []
    for c in range(NCORES):
        zp_r = np.stack([table_arrange(zp0, x[BC * c + r]) for r in range(BC)])
        f_r = np.stack([table_arrange(f0.astype(bf), x[BC * c + r]) for r in range(BC)])
        so_r = np.stack([table_arrange(so0b, x[BC * c + r]) for r in range(BC)])
        in_maps.append(
            {
                "zp0": zp_r,
                "f0": f_r,
                "so0": so_r,
                "wz": wz,
                "wfo": wfo,
                "wd": wdt,
                "bias": bias,
                "decb": decbb,
            }
        )
    return in_maps


def _unpermute(res):
    out = np.empty((B, T, VOCAB), np.float32)
    for c in range(NCORES):
        out[BC * c : BC * (c + 1)] = res[c]["out"]
    return out


def kernel(**inputs) -> np.ndarray:
    from concourse.bass_utils import run_bass_kernel_spmd

    try:
        import jax, tempfile, os

        jax.config.update(
            "jax_compilation_cache_dir",
            os.environ.get("JAX_COMPILATION_CACHE_DIR")
            or os.path.join(tempfile.gettempdir(), "bass_jax_cache"),
        )
    except Exception:
        pass

    nc = _build()
    in_maps = _prep_inputs(inputs)
    res = run_bass_kernel_spmd(nc, in_maps, list(range(NCORES)))
    return _unpermute(res.results)



# revision 2
# speedup vs baseline: 3.5270x; 3.5270x over previous
"""DRNN-Char (4-layer dilated QRNN + decoder) Trainium2 kernel, v4.

Sharding: data-parallel over batch. 16 rows / 8 cores = 2 rows per core.

- Layer 0 is a host-side table lookup (gates depend only on the input token):
  the host sends gathered zp0=(1-f0)*tanh(z0), f0, sigmoid(o0) tables.
- Direct C-space recurrence c = f*c + (1-f)*tanh(z), one tensor_tensor_scan
  per dilation chunk. Scan carry is fp32 internally regardless of operand
  dtype, so all scan operands/outputs are bf16 (measured: scan runs at
  ~2.15ns/elem regardless of dtype; bf16 halves SBUF and speeds the
  surrounding elementwise ops into DVE 2x/4x modes).
- f,o gate matmuls in fp8e4 + DoubleRow; z stays bf16 (z errors are
  first-order in the output; fp8 z measured 5.6e-2 rel err vs 7e-3 all-bf16).
- The inter-layer dilation reorder rides on strided matmul rhs APs
  (measured: no stride penalty on MATMUL).
- fp8 x-copies are ACT-engine Copy-with-scale (ACT is ~2.0us/[128,2048] and
  has slack; GpSimd tensor_scalar measured 29us, DVE 1.2us). x8 converts are
  emitted after the NEXT unit's gate ACTs so ACT never stalls on the DVE
  chain of the current unit.
- zpt = (f-1)*(-tanh) as TS(+(-1), 4x mode, 682ns) + TT(mult, 2x, 1213ns)
  instead of STT (1x, 2279ns).
- decb is added on the host (it's a [V] broadcast over the f32 output).
"""

import numpy as np
import ml_dtypes

EMB = 256
HID = 512
LAYERS = 4
VOCAB = 256
B = 16
T = 2048
NCORES = 8
BC = B // NCORES
HCH = HID // 128

SW = 32.0                      # fp8 weight scale
SX = [32.0, 128.0, 256.0]      # fp8 x scale for h0,h1,h2 (inputs of L1..L3)

_cache = {}


def _build():
    if "nc" in _cache:
        return _cache["nc"]

    import concourse.bass as bass
    import concourse.mybir as mybir
    import concourse.tile as tile
    from concourse import bacc

    f32 = mybir.dt.float32
    bf16 = mybir.dt.bfloat16
    fp8 = mybir.dt.float8e4
    SIG = mybir.ActivationFunctionType.Sigmoid
    TANH = mybir.ActivationFunctionType.Tanh
    COPY = mybir.ActivationFunctionType.Copy
    MULT = mybir.AluOpType.mult
    ADD = mybir.AluOpType.add
    DR = mybir.MatmulPerfMode.DoubleRow

    nc = bacc.Bacc(
        "TRN2",
        target_bir_lowering=False,
        debug=False,
        enable_asserts=False,
        num_devices=NCORES,
    )

    # ---- DRAM inputs (host-prepped, per core) ----
    zp0_d = nc.dram_tensor("zp0", [BC, 4, 128, 4, 512], bf16, kind="ExternalInput").ap()
    f0_d = nc.dram_tensor("f0", [BC, 4, 128, 4, 512], bf16, kind="ExternalInput").ap()
    so0_d = nc.dram_tensor("so0", [BC, 4, 128, 4, 512], bf16, kind="ExternalInput").ap()
    wz_d = nc.dram_tensor("wz", [3, 128, 4, 512], bf16, kind="ExternalInput").ap()
    wfo_d = nc.dram_tensor("wfo", [3, 128, 4, 1024], fp8, kind="ExternalInput").ap()
    wd_d = nc.dram_tensor("wd", [128, 4, VOCAB], bf16, kind="ExternalInput").ap()
    bias_d = nc.dram_tensor("bias", [128, 3, 12], f32, kind="ExternalInput").ap()
    out_d = nc.dram_tensor("out", [BC, T, VOCAB], f32, kind="ExternalOutput").ap()

    with tile.TileContext(nc) as tc:
        with (
            tc.tile_pool(name="consts", bufs=1) as consts,
            tc.tile_pool(name="acts", bufs=1) as acts,
            tc.tile_pool(name="l0t", bufs=2) as l0t,
            tc.tile_pool(name="stage", bufs=2) as stage,
            tc.tile_pool(name="ccl0", bufs=2) as ccl0,
            tc.tile_pool(name="ccp", bufs=2) as ccp,
            tc.tile_pool(name="outs", bufs=2) as outs,
            tc.tile_pool(name="psum", bufs=4, space="PSUM") as psum,
        ):
            # ---- resident tiles ----
            wz_sb = [consts.tile([128, 4, 512], bf16, tag=f"wz{i}", name=f"wz{i}") for i in range(3)]
            wfo_sb = [consts.tile([128, 4, 1024], fp8, tag=f"wfo{i}", name=f"wfo{i}") for i in range(3)]
            wd = consts.tile([128, 4, VOCAB], bf16, tag="wd", name="wd")
            bias = consts.tile([128, 3, 12], f32, tag="bias", name="bias")

            # ping-pong activation buffers per row
            xbuf = [[acts.tile([128, 4, T], bf16, tag=f"x{r}_{p}", name=f"x{r}_{p}")
                     for p in range(2)] for r in range(BC)]
            x8 = [[acts.tile([128, 4, T], fp8, tag=f"x8{r}_{p}", name=f"x8{r}_{p}")
                   for p in range(2)] for r in range(BC)]

            # ---- const DMAs ----
            for i in range(3):
                nc.gpsimd.dma_start(wz_sb[i][:], wz_d[i])
                nc.gpsimd.dma_start(wfo_sb[i][:], wfo_d[i])
            nc.gpsimd.dma_start(wd[:], wd_d[:])
            nc.gpsimd.dma_start(bias[:], bias_d[:])

            # ---- layer 0: scan over host-gathered tables ----
            # xbuf[r][0] gets h0 (natural time order); x8[r][0] = h0 * SX0 (ACT).
            for r in range(BC):
                prev = [None] * HCH
                for q4 in range(4):
                    zp = l0t.tile([128, 4, 512], bf16, tag="zp", name="zp")
                    f0t = l0t.tile([128, 4, 512], bf16, tag="f0", name="f0")
                    so0t = l0t.tile([128, 4, 512], bf16, tag="so0", name="so0")
                    nc.sync.dma_start(zp[:], zp0_d[r, q4])
                    nc.sync.dma_start(f0t[:], f0_d[r, q4])
                    nc.sync.dma_start(so0t[:], so0_d[r, q4])
                    sl = slice(q4 * 512, q4 * 512 + 512)
                    for h in range(HCH):
                        cq = ccl0.tile([128, 512], bf16, tag=f"cc0_{h}", name=f"cc0_{h}")
                        init = 0.0 if q4 == 0 else prev[h][:, 511:512]
                        nc.vector.tensor_tensor_scan(
                            cq[:], f0t[:, h, :], zp[:, h, :],
                            initial=init, op0=MULT, op1=ADD,
                        )
                        prev[h] = cq
                        nc.vector.tensor_tensor(
                            xbuf[r][0][:, h, sl], so0t[:, h, :], cq[:], MULT
                        )
                for h in range(HCH):
                    nc.scalar.activation(
                        x8[r][0][:, h, :], xbuf[r][0][:, h, :], COPY,
                        bias=0.0, scale=SX[0],
                    )

            # ---- layers 1..3 ----
            # pending_conv holds (out_ap, in_ap, scale) fp8 converts to emit on
            # the ACT queue after the NEXT unit's gate activations.
            pending_conv = []

            def flush_conv():
                while pending_conv:
                    o_ap, i_ap, s = pending_conv.pop(0)
                    nc.scalar.activation(o_ap, i_ap, COPY, bias=0.0, scale=s)

            cur = 0
            for li in (1, 2, 3):
                idx = li - 1
                rho = 2 ** li
                ascale = 1.0 / (SW * SX[idx])
                nxt = 1 - cur
                for r in range(BC):
                    xin, x8in = xbuf[r][cur], x8[r][cur]
                    xout, x8out = xbuf[r][nxt], x8[r][nxt]
                    for h in range(HCH):
                        # z gate: bf16; ztneg = tanh(-(pre + bz))
                        zt = stage.tile([128, T], bf16, tag="zt", name="zt")
                        for pb in range(2):
                            ps = psum.tile([128, 1024], f32, tag="ps", name="ps")
                            for k in range(4):
                                for u in range(2):
                                    nc.tensor.matmul(
                                        ps[:, u * 512 : (u + 1) * 512],
                                        lhsT=wz_sb[idx][:, k, h * 128 : (h + 1) * 128],
                                        rhs=xin[:, k, pb + u * 1024 : pb + u * 1024 + 1023 : 2],
                                        start=(k == 0),
                                        stop=(k == 3),
                                    )
                            nc.scalar.activation(
                                zt[:, pb * 1024 : (pb + 1) * 1024], ps[:], TANH,
                                bias=bias[:, idx, h : h + 1], scale=-1.0,
                            )
                        # f,o gates: fp8 DoubleRow, bf16 out
                        gts = {"z": zt}
                        for g, gname in ((0, "f"), (1, "o")):
                            gt = stage.tile([128, T], bf16, tag=gname, name=gname)
                            for pb in range(2):
                                ps = psum.tile([128, 1024], f32, tag="ps", name="ps")
                                for kp in range(2):
                                    for u in range(2):
                                        nc.tensor.matmul(
                                            ps[:, u * 512 : (u + 1) * 512],
                                            lhsT=wfo_sb[idx][:, 2 * kp : 2 * kp + 2, g * 512 + h * 128 : g * 512 + (h + 1) * 128],
                                            rhs=x8in[:, 2 * kp : 2 * kp + 2, pb + u * 1024 : pb + u * 1024 + 1023 : 2],
                                            start=(kp == 0),
                                            stop=(kp == 1),
                                            perf_mode=DR,
                                        )
                                nc.scalar.activation(
                                    gt[:, pb * 1024 : (pb + 1) * 1024], ps[:], SIG,
                                    bias=bias[:, idx, (g + 1) * 4 + h : (g + 1) * 4 + h + 1],
                                    scale=ascale,
                                )
                            gts[gname] = gt
                        # previous unit's fp8 convert goes behind these gates
                        flush_conv()
                        # DVE chain: fm1 = f - 1 (TS 4x); zpt = fm1 * ztneg (TT 2x)
                        fm1 = ccp.tile([128, T], bf16, tag="fm1", name="fm1")
                        nc.vector.tensor_scalar_add(fm1[:], gts["f"][:], -1.0)
                        zpt = ccp.tile([128, T], bf16, tag="zp", name="zp")
                        nc.vector.tensor_tensor(zpt[:], fm1[:], gts["z"][:], MULT)
                        cc = ccp.tile([128, T], bf16, tag="cc", name="cc")
                        for j in range(rho):
                            ssl = slice(j * (T // rho), (j + 1) * (T // rho))
                            nc.vector.tensor_tensor_scan(
                                cc[:, ssl], gts["f"][:, ssl], zpt[:, ssl],
                                initial=0.0, op0=MULT, op1=ADD,
                            )
                        nc.vector.tensor_tensor(xout[:, h, :], gts["o"][:], cc[:], MULT)
                        if li < 3:
                            pending_conv.append(
                                (x8out[:, h, :], xout[:, h, :], SX[li])
                            )
                cur = nxt
            flush_conv()

            # ---- decoder (h3 in dilation-8 order; scatter rows on DMA out) ----
            for r in range(BC):
                xin = xbuf[r][cur]
                for mt in range(T // 128):
                    ps = psum.tile([128, 1024], f32, tag="ps", name="ps")
                    for k in range(4):
                        nc.tensor.matmul(
                            ps[:, 0:VOCAB],
                            lhsT=xin[:, k, mt * 128 : (mt + 1) * 128],
                            rhs=wd[:, k, :],
                            start=(k == 0),
                            stop=(k == 3),
                        )
                    ot = outs.tile([128, VOCAB], f32, tag="ot", name="ot")
                    nc.scalar.activation(ot[:], ps[:, 0:VOCAB], COPY, bias=0.0, scale=1.0)
                    # dilation-8 index i = j*256 + q -> t = 8q + j
                    t0 = 1024 * (mt % 2) + mt // 2
                    nc.sync.dma_start(out_d[r, t0 : t0 + 1017 : 8, :], ot[:])

    nc.compile()
    _cache["nc"] = nc
    return nc


def _prep_inputs(inputs):
    bf = ml_dtypes.bfloat16
    f8 = ml_dtypes.float8_e4m3fn
    x = np.asarray(inputs["x"]).astype(np.int64)
    emb = np.asarray(inputs["emb"], dtype=np.float32)
    Ws = [np.asarray(inputs[f"W{i}"], dtype=np.float32) for i in range(LAYERS)]
    bs = [np.asarray(inputs[f"b{i}"], dtype=np.float32) for i in range(LAYERS)]
    decW = np.asarray(inputs["decW"], dtype=np.float32)

    # layer-0 per-vocab gate tables
    pre0 = emb @ Ws[0] + bs[0]          # [VOCAB, 3H]
    zt0 = np.tanh(pre0[:, :HID])
    f0 = 1.0 / (1.0 + np.exp(-pre0[:, HID : 2 * HID]))
    so0 = 1.0 / (1.0 + np.exp(-pre0[:, 2 * HID :]))
    f0 = f0.astype(bf).astype(np.float32)
    zp0 = ((1.0 - f0) * zt0).astype(bf)
    so0b = so0.astype(bf)

    def table_arrange(tab, idx):
        # tab [VOCAB, HID] -> gathered [T, HID] -> [4(q), 128, 4(k), 512]
        g = tab[idx]                                  # [T, 512]
        return np.ascontiguousarray(
            g.T.reshape(4, 128, 4, 512).transpose(2, 1, 0, 3)
        )

    wz = np.stack(
        [np.ascontiguousarray(Ws[i][:, :HID].reshape(4, 128, 512).transpose(1, 0, 2)).astype(bf) for i in range(1, 4)]
    )
    wfo = np.stack(
        [
            np.ascontiguousarray((Ws[i][:, HID:] * SW).reshape(4, 128, 1024).transpose(1, 0, 2)).astype(f8)
            for i in range(1, 4)
        ]
    )
    wdt = np.ascontiguousarray(decW.reshape(4, 128, VOCAB).transpose(1, 0, 2)).astype(bf)

    bias = np.zeros((128, 3, 12), np.float32)
    for i in range(1, 4):
        bb = bs[i].reshape(3, 4, 128)  # [gate, h, p]
        bias[:, i - 1, 0:4] = -bb[0].T
        bias[:, i - 1, 4:8] = bb[1].T
        bias[:, i - 1, 8:12] = bb[2].T

    in_maps = []
    for c in range(NCORES):
        zp_r = np.stack([table_arrange(zp0, x[BC * c + r]) for r in range(BC)])
        f_r = np.stack([table_arrange(f0.astype(bf), x[BC * c + r]) for r in range(BC)])
        so_r = np.stack([table_arrange(so0b, x[BC * c + r]) for r in range(BC)])
        in_maps.append(
            {
                "zp0": zp_r,
                "f0": f_r,
                "so0": so_r,
                "wz": wz,
                "wfo": wfo,
                "wd": wdt,
                "bias": bias,
            }
        )
    return in_maps


def _unpermute(res, decb):
    out = np.empty((B, T, VOCAB), np.float32)
    for c in range(NCORES):
        out[BC * c : BC * (c + 1)] = res[c]["out"]
    out += decb.reshape(1, 1, VOCAB)
    return out


def kernel(**inputs) -> np.ndarray:
    from concourse.bass_utils import run_bass_kernel_spmd

    try:
        import jax, tempfile, os

        jax.config.update(
            "jax_compilation_cache_dir",
            os.environ.get("JAX_COMPILATION_CACHE_DIR")
            or os.path.join(tempfile.gettempdir(), "bass_jax_cache"),
        )
    except Exception:
        pass

    nc = _build()
    in_maps = _prep_inputs(inputs)
    res = run_bass_kernel_spmd(nc, in_maps, list(range(NCORES)))
    decb = np.asarray(inputs["decb"], dtype=np.float32)
    return _unpermute(res.results, decb)
